# revision 30
# baseline (speedup 1.0000x reference)
"""Trainium2 Bass kernel for nn_MessagePassing_7937099563205 (GNN message passing).

Computes out[n, k] = sum_{e : src[e] == n} edge_attrs.flat[k*E + e]
(i.e. jax.ops.segment_sum of edge_attrs.reshape(-1).reshape(F, E).T over
attr_idx[0]) for E=4M edges, F=16 features, N=100000 nodes, on 8 NeuronCores.

Strategy (PE-matmul segment sum; no scatter, no indices on device):
  Host:   sort nodes by degree (desc), deal round-robin to the 8 cores so all
          cores share one schedule; pad each node's edge list to groups of
          G=8; pack one group as a 128-row fp16 column (row = feat*8 + slot).
          Columns are ordered (block of 512 nodes) x (round r) x (node),
          where round r holds each live node's r-th group — so a node's
          groups all land in the SAME column position of its block across
          rounds.
  Device: stream the column array (dense contiguous DMA). A constant
          block-diagonal ones matrix O[128, 16] (O[k, m] = 1 iff k//8 == m)
          is the stationary matmul operand: psum[m, col] += sum_i
          vals[8m+i, col].  Accumulating R rounds in PSUM yields COMPLETE
          per-node feature sums. Evict psum[16, 512] per block via DVE and
          DMA to a dense output table.
  Host:   invert the node permutation, trim to N.
"""

import sys
import numpy as np

_REPO = "/opt/trn_rl_repo"
if _REPO not in sys.path:
    sys.path.append(_REPO)

# ---------------------------------------------------------------- config ----

E = 4_000_000
F = 16
N = 100_000
NC = 8                      # cores
G = 8                       # edges per group (one psum contraction)
BLK = 512                   # nodes per block (= psum bank columns)
NB = 25                     # blocks per core (8*25*512 = 102400 >= N)
NPC = NB * BLK              # node positions per core
NPAD = NC * NPC

_PROGRAM_CACHE: dict = {}


# ------------------------------------------------------------ the program ---

def build_program(ncols, blk=BLK, f=F):
    """ncols: tuple of per-block tuples; ncols[b][r] = live columns of round r.

    SPMD-identical across cores (schedule is the max over cores; dead
    columns hold zeros).
    """
    import concourse.bacc as bacc
    import concourse.mybir as mybir
    from concourse import bass, tile

    nb = len(ncols)
    total = sum(sum(rs) for rs in ncols)
    maxc = max(sum(rs) for rs in ncols)
    nc = bacc.Bacc(None)
    vals = nc.declare_dram_parameter("vals", [128, total], mybir.dt.float16,
                                     isOutput=False)
    ones = nc.declare_dram_parameter("ones", [128, f], mybir.dt.float16,
                                     isOutput=False)
    out = nc.declare_dram_parameter("out", [nb * f, blk], mybir.dt.float16,
                                    isOutput=True)

    with tile.TileContext(nc) as tc:
        with tc.tile_pool(name="misc", bufs=1) as misc, \
             tc.tile_pool(name="blocks", bufs=12) as blocks, \
             tc.tile_pool(name="psum", bufs=7, space=bass.MemorySpace.PSUM) \
                as psum, \
             tc.tile_pool(name="warm", bufs=1, space=bass.MemorySpace.PSUM) \
                as warmp, \
             tc.tile_pool(name="outs", bufs=6) as outs:
            # ones on the scalar queue so block 0's value DMA leads sync
            ot = misc.tile([128, f], mybir.dt.float16)
            nc.scalar.dma_start(ot[:], ones[:])

            # PE warm-up until the first value block lands (~16us): dummy
            # matmuls keep the HAM clock gate at 8/8 so real matmuls never
            # run at the cold 1.2 GHz clock. Uses memset weights so it has
            # no DMA dependency at all.
            wsrc = misc.tile([128, 128], mybir.dt.float16)
            wones = misc.tile([128, f], mybir.dt.float16)
            nc.vector.memset(wsrc[:], 0.0)
            nc.vector.memset(wones[:], 0.0)
            wps = warmp.tile([f, 128], mybir.dt.float32)
            for _ in range(88):
                nc.tensor.matmul(wps[:], wones[:], wsrc[:], start=True,
                                 stop=True)

            # per-block value DMAs, alternating between the two HWDGE
            # dispatch queues (Sync / Scalar) so dispatch bubbles on one
            # sequencer don't starve the DMA engines.
            # evictions batched OB full blocks per out tile -> fewer
            # out-DMA dispatches contending with the value stream
            OB = 8
            off = 0
            so = None
            nfull = 0
            for b in range(nb):
                cb = sum(ncols[b])
                vq = nc.sync if b % 2 == 0 else nc.scalar
                oq = nc.scalar if b % 2 == 0 else nc.sync
                t = blocks.tile([128, maxc], mybir.dt.float16, tag="blk")
                vq.dma_start(t[:, :cb], vals[:, off:off + cb])
                ps = psum.tile([f, blk], mybir.dt.float32, tag="ps")
                o2 = 0
                for r, n in enumerate(ncols[b]):
                    nc.tensor.matmul(ps[:, :n], ot[:], t[:, o2:o2 + n],
                                     start=(r == 0),
                                     stop=(r == len(ncols[b]) - 1))
                    o2 += n
                w = ncols[b][0] if ncols[b] else 0
                if w == blk:
                    if nfull == 0:
                        so = outs.tile([f, OB * blk], mybir.dt.float16,
                                       tag="so")
                        sb = b
                    nc.vector.tensor_copy(so[:, nfull * blk:(nfull + 1) * blk],
                                          ps[:])
                    nfull += 1
                    flush = (nfull == OB or b == nb - 1
                             or (ncols[b + 1][0] if ncols[b + 1] else 0) != blk)
                    if flush:
                        oq.dma_start(
                            out[sb * f:(sb + nfull) * f, :].rearrange(
                                "(c p) q -> p c q", p=f),
                            so[:, :nfull * blk].rearrange(
                                "p (c q) -> p c q", q=blk))
                        nfull = 0
                elif w:
                    sp = outs.tile([f, blk], mybir.dt.float16, tag="sp")
                    nc.vector.tensor_copy(sp[:, :w], ps[:, :w])
                    oq.dma_start(out[b * f:(b + 1) * f, :w], sp[:, :w])
                off += cb

    nc.finalize()
    return nc


def get_program(ncols):
    key = tuple(tuple(rs) for rs in ncols)
    if key not in _PROGRAM_CACHE:
        _PROGRAM_CACHE[key] = build_program(key)
    return _PROGRAM_CACHE[key]


# ------------------------------------------------------- host preprocessing --

def preprocess(edge_attrs, attr_idx, e=E, f=F, n=N, n_cores=NC, g=G,
               blk=BLK, nb=NB):
    """Build per-core fp16 column arrays + the shared round schedule.

    Returns (in_maps, ncols, nodes_pc) where in_maps[c]["vals"] is
    (128, TOTAL) fp16, ncols[b][r] = live columns in round r of block b,
    nodes_pc[c, j] = node id at position j of core c.
    """
    npc = nb * blk
    npad = n_cores * npc
    ea = np.asarray(edge_attrs, dtype=np.float32).reshape(e, f)
    EA2 = ea.reshape(f, e)                      # EA2[k, e] = flat[k*E + e]
    src = np.asarray(attr_idx)[0].astype(np.int64)

    deg = np.zeros(npad, np.int64)
    deg[:n] = np.bincount(src, minlength=n)
    order_nodes = np.argsort(-deg, kind="stable")
    nodes_pc = np.stack([order_nodes[c::n_cores] for c in range(n_cores)])
    deg_pc = deg[nodes_pc]                      # (NC, NPC), desc per row
    grp = -(-deg_pc // g)                       # groups per position
    # real nodes always get >= 1 group (so their psum column is written);
    # padding ids (>= n, all at the tail) get 0 and cost no columns.
    grp[(nodes_pc < n) & (grp == 0)] = 1
    Gmax = grp.max(axis=0)                      # (NPC,), non-increasing
    Gb = Gmax.reshape(nb, blk)
    ncols = tuple(tuple(int((Gb[b] > r).sum()) for r in range(int(Gb[b, 0])))
                  for b in range(nb))

    # column order: block b, round r, live position j (prefix of block)
    pos_list = np.concatenate(
        [blk * b + np.arange(nr, dtype=np.int64)
         for b, rs in enumerate(ncols) for nr in rs])
    rnd_list = np.concatenate(
        [np.full(nr, r, np.int64) for rs in ncols for r, nr in enumerate(rs)])
    T = len(pos_list)

    order_e = np.argsort(src, kind="stable").astype(np.int64)
    cum = np.concatenate(([0], np.cumsum(deg)))  # len npad+1

    in_maps = []
    ones = np.zeros((128, f), np.float16)
    for m in range(f):
        ones[m * g:(m + 1) * g, m] = 1.0
    for c in range(n_cores):
        node = nodes_pc[c, pos_list]             # (T,)
        base = cum[node] + g * rnd_list
        eidx = base[:, None] + np.arange(g)[None, :]
        valid = eidx < cum[node + 1][:, None]
        eg = order_e[np.where(valid, eidx, 0)]   # (T, g)
        Vt = EA2[:, eg.ravel()].reshape(f, T, g)
        Vt[:, ~valid] = 0.0
        V = np.ascontiguousarray(
            Vt.transpose(0, 2, 1).reshape(128, T).astype(np.float16))
        in_maps.append({"vals": V, "ones": ones})
    return in_maps, ncols, nodes_pc


def postprocess(results, nodes_pc, n=N, f=F, blk=BLK, nb=NB, n_cores=NC):
    npad = n_cores * nb * blk
    full = np.zeros((npad, f), np.float32)
    for c in range(n_cores):
        o = np.asarray(results[c]["out"], np.float32)
        # (NB*f, BLK) -> (NB, f, BLK) -> (NB, BLK, f) -> (NPC, f)
        pc = o.reshape(nb, f, blk).transpose(0, 2, 1).reshape(nb * blk, f)
        full[nodes_pc[c]] = pc
    return np.ascontiguousarray(full[:n])


# ---------------------------------------------------------------- kernel ----

def kernel(edge_attrs=None, attr_idx=None, n_nodes=None, **_ignored):
    from concourse.bass_utils import run_bass_kernel_spmd

    in_maps, ncols, nodes_pc = preprocess(edge_attrs, attr_idx)
    ncp = get_program(ncols)
    res = run_bass_kernel_spmd(ncp, in_maps, core_ids=list(range(NC)))
    return postprocess(res.results, nodes_pc)


# revision 31
# speedup vs baseline: 1.2319x; 1.2319x over previous
"""Trainium2 Bass kernel for nn_MessagePassing_7937099563205 (GNN message passing).

Computes out[n, k] = sum_{e : src[e] == n} edge_attrs.flat[k*E + e]
(i.e. jax.ops.segment_sum of edge_attrs.reshape(-1).reshape(F, E).T over
attr_idx[0]) for E=4M edges, F=16 features, N=100000 nodes, on 8 NeuronCores.

Strategy (PE-matmul segment sum; no scatter, no indices on device):
  Host:   sort nodes by degree (desc), deal round-robin to the 8 cores so all
          cores share one schedule; pad each node's edge list to groups of
          G=8; pack one group as a 128-row fp16 column (row = feat*8 + slot).
          Columns are ordered (block of 512 nodes) x (round r) x (node),
          where round r holds each live node's r-th group — so a node's
          groups all land in the SAME column position of its block across
          rounds.
  Device: stream the column array (dense contiguous DMA). A constant
          block-diagonal ones matrix O[128, 16] (O[k, m] = 1 iff k//8 == m)
          is the stationary matmul operand: psum[m, col] += sum_i
          vals[8m+i, col].  Accumulating R rounds in PSUM yields COMPLETE
          per-node feature sums. Evict psum[16, 512] per block via DVE and
          DMA to a dense output table.
  Host:   invert the node permutation, trim to N.
"""

import sys
import numpy as np

_REPO = "/opt/trn_rl_repo"
if _REPO not in sys.path:
    sys.path.append(_REPO)

# ---------------------------------------------------------------- config ----

E = 4_000_000
F = 16
N = 100_000
NC = 8                      # cores
G = 8                       # edges per group (one psum contraction)
BLK = 512                   # nodes per block (= psum bank columns)
NB = 25                     # blocks per core (8*25*512 = 102400 >= N)
NPC = NB * BLK              # node positions per core
NPAD = NC * NPC

_PROGRAM_CACHE: dict = {}


# ------------------------------------------------------------ the program ---

def build_program(ncols, blk=BLK, f=F):
    """ncols: tuple of per-block tuples; ncols[b][r] = live columns of round r.

    SPMD-identical across cores (schedule is the max over cores; dead
    columns hold zeros).
    """
    import concourse.bacc as bacc
    import concourse.mybir as mybir
    from concourse import bass, tile

    nb = len(ncols)
    total = sum(sum(rs) for rs in ncols)
    maxc = max(sum(rs) for rs in ncols)
    nc = bacc.Bacc(None)
    vals = nc.declare_dram_parameter("vals", [128, total], mybir.dt.float16,
                                     isOutput=False)
    ones = nc.declare_dram_parameter("ones", [128, f], mybir.dt.float16,
                                     isOutput=False)
    out = nc.declare_dram_parameter("out", [nb * f, blk], mybir.dt.float16,
                                    isOutput=True)

    with tile.TileContext(nc) as tc:
        with tc.tile_pool(name="misc", bufs=1) as misc, \
             tc.tile_pool(name="blocks", bufs=12) as blocks, \
             tc.tile_pool(name="psum", bufs=7, space=bass.MemorySpace.PSUM) \
                as psum, \
             tc.tile_pool(name="warm", bufs=1, space=bass.MemorySpace.PSUM) \
                as warmp, \
             tc.tile_pool(name="outs", bufs=6) as outs:
            # ones on the scalar queue so block 0's value DMA leads sync
            ot = misc.tile([128, f], mybir.dt.float16)
            nc.scalar.dma_start(ot[:], ones[:])

            # PE warm-up until the first value block lands (~16us): dummy
            # matmuls keep the HAM clock gate at 8/8 so real matmuls never
            # run at the cold 1.2 GHz clock. Uses memset weights so it has
            # no DMA dependency at all.
            wsrc = misc.tile([128, 128], mybir.dt.float16)
            wones = misc.tile([128, f], mybir.dt.float16)
            nc.vector.memset(wsrc[:], 0.0)
            nc.vector.memset(wones[:], 0.0)
            wps = warmp.tile([f, 128], mybir.dt.float32)
            for _ in range(88):
                nc.tensor.matmul(wps[:], wones[:], wsrc[:], start=True,
                                 stop=True)

            # per-block value DMAs, alternating between the two HWDGE
            # dispatch queues (Sync / Scalar) so dispatch bubbles on one
            # sequencer don't starve the DMA engines.
            # evictions batched OB full blocks per out tile -> fewer
            # out-DMA dispatches contending with the value stream
            OB = 4
            off = 0
            so = None
            nfull = 0
            for b in range(nb):
                cb = sum(ncols[b])
                vq = nc.sync if b % 2 == 0 else nc.scalar
                oq = nc.scalar if b % 2 == 0 else nc.sync
                t = blocks.tile([128, maxc], mybir.dt.float16, tag="blk")
                vq.dma_start(t[:, :cb], vals[:, off:off + cb])
                ps = psum.tile([f, blk], mybir.dt.float32, tag="ps")
                o2 = 0
                for r, n in enumerate(ncols[b]):
                    nc.tensor.matmul(ps[:, :n], ot[:], t[:, o2:o2 + n],
                                     start=(r == 0),
                                     stop=(r == len(ncols[b]) - 1))
                    o2 += n
                w = ncols[b][0] if ncols[b] else 0
                if w == blk:
                    if nfull == 0:
                        so = outs.tile([f, OB * blk], mybir.dt.float16,
                                       tag="so")
                        sb = b
                    nc.vector.tensor_copy(so[:, nfull * blk:(nfull + 1) * blk],
                                          ps[:])
                    nfull += 1
                    flush = (nfull == OB or b == nb - 1
                             or (ncols[b + 1][0] if ncols[b + 1] else 0) != blk)
                    if flush:
                        oq.dma_start(
                            out[sb * f:(sb + nfull) * f, :].rearrange(
                                "(c p) q -> p c q", p=f),
                            so[:, :nfull * blk].rearrange(
                                "p (c q) -> p c q", q=blk))
                        nfull = 0
                elif w:
                    sp = outs.tile([f, blk], mybir.dt.float16, tag="sp")
                    nc.vector.tensor_copy(sp[:, :w], ps[:, :w])
                    oq.dma_start(out[b * f:(b + 1) * f, :w], sp[:, :w])
                off += cb

    nc.finalize()
    return nc


def get_program(ncols):
    key = tuple(tuple(rs) for rs in ncols)
    if key not in _PROGRAM_CACHE:
        _PROGRAM_CACHE[key] = build_program(key)
    return _PROGRAM_CACHE[key]


# ------------------------------------------------------- host preprocessing --

def preprocess(edge_attrs, attr_idx, e=E, f=F, n=N, n_cores=NC, g=G,
               blk=BLK, nb=NB):
    """Build per-core fp16 column arrays + the shared round schedule.

    Returns (in_maps, ncols, nodes_pc) where in_maps[c]["vals"] is
    (128, TOTAL) fp16, ncols[b][r] = live columns in round r of block b,
    nodes_pc[c, j] = node id at position j of core c.
    """
    npc = nb * blk
    npad = n_cores * npc
    ea = np.asarray(edge_attrs, dtype=np.float32).reshape(e, f)
    EA2 = ea.reshape(f, e)                      # EA2[k, e] = flat[k*E + e]
    src = np.asarray(attr_idx)[0].astype(np.int64)

    deg = np.zeros(npad, np.int64)
    deg[:n] = np.bincount(src, minlength=n)
    order_nodes = np.argsort(-deg, kind="stable")
    nodes_pc = np.stack([order_nodes[c::n_cores] for c in range(n_cores)])
    deg_pc = deg[nodes_pc]                      # (NC, NPC), desc per row
    grp = -(-deg_pc // g)                       # groups per position
    # real nodes always get >= 1 group (so their psum column is written);
    # padding ids (>= n, all at the tail) get 0 and cost no columns.
    grp[(nodes_pc < n) & (grp == 0)] = 1
    Gmax = grp.max(axis=0)                      # (NPC,), non-increasing
    Gb = Gmax.reshape(nb, blk)
    ncols = tuple(tuple(int((Gb[b] > r).sum()) for r in range(int(Gb[b, 0])))
                  for b in range(nb))

    # column order: block b, round r, live position j (prefix of block)
    pos_list = np.concatenate(
        [blk * b + np.arange(nr, dtype=np.int64)
         for b, rs in enumerate(ncols) for nr in rs])
    rnd_list = np.concatenate(
        [np.full(nr, r, np.int64) for rs in ncols for r, nr in enumerate(rs)])
    T = len(pos_list)

    order_e = np.argsort(src, kind="stable").astype(np.int64)
    cum = np.concatenate(([0], np.cumsum(deg)))  # len npad+1

    in_maps = []
    ones = np.zeros((128, f), np.float16)
    for m in range(f):
        ones[m * g:(m + 1) * g, m] = 1.0
    for c in range(n_cores):
        node = nodes_pc[c, pos_list]             # (T,)
        base = cum[node] + g * rnd_list
        eidx = base[:, None] + np.arange(g)[None, :]
        valid = eidx < cum[node + 1][:, None]
        eg = order_e[np.where(valid, eidx, 0)]   # (T, g)
        Vt = EA2[:, eg.ravel()].reshape(f, T, g)
        Vt[:, ~valid] = 0.0
        V = np.ascontiguousarray(
            Vt.transpose(0, 2, 1).reshape(128, T).astype(np.float16))
        in_maps.append({"vals": V, "ones": ones})
    return in_maps, ncols, nodes_pc


def postprocess(results, nodes_pc, n=N, f=F, blk=BLK, nb=NB, n_cores=NC):
    npad = n_cores * nb * blk
    full = np.zeros((npad, f), np.float32)
    for c in range(n_cores):
        o = np.asarray(results[c]["out"], np.float32)
        # (NB*f, BLK) -> (NB, f, BLK) -> (NB, BLK, f) -> (NPC, f)
        pc = o.reshape(nb, f, blk).transpose(0, 2, 1).reshape(nb * blk, f)
        full[nodes_pc[c]] = pc
    return np.ascontiguousarray(full[:n])


# ---------------------------------------------------------------- kernel ----

def kernel(edge_attrs=None, attr_idx=None, n_nodes=None, **_ignored):
    from concourse.bass_utils import run_bass_kernel_spmd

    in_maps, ncols, nodes_pc = preprocess(edge_attrs, attr_idx)
    ncp = get_program(ncols)
    res = run_bass_kernel_spmd(ncp, in_maps, core_ids=list(range(NC)))
    return postprocess(res.results, nodes_pc)
